# revision 15
# baseline (speedup 1.0000x reference)
"""Trainium2 Bass kernel for the 3-group sparse attention module.

Shapes: x [4, 1024, 768], H=8 heads, head_dim 96 split into 3 groups of 32.
  qkv = x @ W_qkv -> q,k,v [B,H,N,96]; groups q3..q5/k3..k5/v3..v5 (32 each)
  x3 = attend(q4, [k3,k4], [v3,v4]); x4 = attend(q5, [k3,k5], [v3,v5])
  x5 = attend(q5, [k4,k5], [v4,v5]);  out = [x3|x4|x5] @ W_proj + b_proj
  scale = 96 ** -0.5

Sharding: 8 cores = 4 batches x 2 query-halves (no collectives).  Each core
computes k/v for the full sequence of its batch (all 8 heads) but queries /
attention / projection only for its 512 rows.  Host passes x transposed
(bf16) with the core's query rows first, so the SPMD graph is identical on
every core; key/value row order is consistently permuted which leaves
attention outputs unchanged.

Everything on-chip runs in "transposed activation space":
  qT/kT[d, n] from matmul(lhsT=W chunk, rhs=xT);  v[m, d] natural.
  S^T[m, n] = matmul(lhsT=kT[32, m-tile], rhs=qT[32, nq]) -- K=32 row-tiled.
  E = exp(scale * S^T) on ScalarE straight out of PSUM (scores are provably
  small: |s*scale| < ~1.2, so no max-subtraction pass is needed).
  y^T[d, n] = matmul(lhsT=[v|1][m-tile, 33], rhs=E) accumulated over m;
  row 32 of the PSUM then holds the softmax denominator Z for free.
  exp(q5 k5^T) @ [v5|1] is shared between x4 and x5 (computed once).
  proj: out[n, :] = matmul(lhsT=yT chunk, rhs=W_proj chunk) + bias.
"""

import numpy as np
import ml_dtypes

B, N, C, H = 4, 1024, 768, 8
HD = 96          # head dim
G = 32           # group dim
NQ = 512         # query rows per core
SCALE = float(HD) ** -0.5
P = 128
NCORES = 8

_CACHE = {}
AV_COL_TILING = False
AV_INTERLEAVE = False
ST_INTERLEAVE = False


def _build_graph():
    import concourse.bass as bass
    import concourse.tile as tile
    from concourse import bacc, mybir

    f32 = mybir.dt.float32
    bf16 = mybir.dt.bfloat16

    nc = bacc.Bacc()

    xt_d = nc.declare_dram_parameter("xt", [C, N], bf16, isOutput=False)
    wq_d = nc.declare_dram_parameter("wq", [C, 768], bf16, isOutput=False)
    wk_d = nc.declare_dram_parameter("wk", [C, 768], bf16, isOutput=False)
    wv_d = nc.declare_dram_parameter("wv", [C, 768], bf16, isOutput=False)
    wp_d = nc.declare_dram_parameter("wp", [C, C], bf16, isOutput=False)
    bias_d = nc.declare_dram_parameter("bias", [P, C], f32, isOutput=False)
    out_d = nc.declare_dram_parameter("out", [NQ, C], f32, isOutput=True)

    CH = C // P  # 6 chunks of 128 along the contraction/channel dims

    with tile.TileContext(nc) as tc:
        with (
            tc.tile_pool(name="wgt", bufs=1) as wgt,
            tc.tile_pool(name="acts", bufs=1) as acts,
            tc.tile_pool(name="epool", bufs=24) as epool,
            tc.tile_pool(name="small", bufs=2) as small,
            tc.tile_pool(name="outp", bufs=2) as outp,
            tc.tile_pool(name="psA", bufs=2, space="PSUM") as psA,
            tc.tile_pool(name="psB", bufs=4, space="PSUM") as psB,
        ):
            # ---- stage inputs in SBUF ----
            xt = [wgt.tile([P, N], bf16, name=f"xt{i}") for i in range(CH)]
            wq = [wgt.tile([P, 768], bf16, name=f"wq{i}") for i in range(CH)]
            wk = [wgt.tile([P, 768], bf16, name=f"wk{i}") for i in range(CH)]
            wv = [wgt.tile([P, 768], bf16, name=f"wv{i}") for i in range(CH)]
            wp = [wgt.tile([P, C], bf16, name=f"wp{i}") for i in range(CH)]
            bias = wgt.tile([P, C], f32, name="bias")
            for i in range(CH):
                nc.sync.dma_start(xt[i][:], xt_d[P * i:P * (i + 1), :])
                nc.sync.dma_start(wq[i][:], wq_d[P * i:P * (i + 1), :])
                nc.sync.dma_start(wk[i][:], wk_d[P * i:P * (i + 1), :])
                nc.sync.dma_start(wv[i][:], wv_d[P * i:P * (i + 1), :])
                nc.sync.dma_start(wp[i][:], wp_d[P * i:P * (i + 1), :])
            nc.sync.dma_start(bias[:], bias_d[:])

            # ---- persistent activation tensors ----
            # qT: [768, 512]  per head h (96 rows at 96h): [q4; q5; q5]
            q_sb = [acts.tile([P, NQ], bf16, name=f"q{i}") for i in range(CH)]
            # kT: [768, 1024] per head: [k3; k5; k4]
            k_sb = [acts.tile([P, N], bf16, name=f"k{i}") for i in range(CH)]
            # kT copy #2 per head: [k4; k3; -] (for the wave-2 score blocks)
            k2_sb = [acts.tile([P, N], bf16, name=f"k2_{i}") for i in range(CH)]
            # v natural per m-tile: 24 groups of [v_g | 1] (33 cols each)
            v_sb = [acts.tile([P, 24 * 33], bf16, name=f"v{i}") for i in range(8)]
            # unnormalized y^T (bf16) channels: 256*g + 32h + d
            u_sb = [acts.tile([P, NQ], bf16, name=f"u{i}") for i in range(CH)]

            def band(h, j):
                """(tensor index, partition offset) of 32-row band j of head h."""
                p = 96 * h + 32 * j
                return p // P, p % P

            # ---- qT / kT generation ----
            for co in range(CH):
                ps = psA.tile([P, NQ], f32, tag="A")
                for ci in range(CH):
                    nc.tensor.matmul(
                        ps[:], lhsT=wq[ci][:, P * co:P * (co + 1)],
                        rhs=xt[ci][:, 0:NQ],
                        start=(ci == 0), stop=(ci == CH - 1))
                nc.vector.tensor_copy(q_sb[co][:], ps[:])
            for co in range(CH):
                for nh in range(2):
                    ps = psA.tile([P, NQ], f32, tag="A")
                    for ci in range(CH):
                        nc.tensor.matmul(
                            ps[:], lhsT=wk[ci][:, P * co:P * (co + 1)],
                            rhs=xt[ci][:, NQ * nh:NQ * (nh + 1)],
                            start=(ci == 0), stop=(ci == CH - 1))
                    nc.vector.tensor_copy(k_sb[co][:, NQ * nh:NQ * (nh + 1)], ps[:])

            # k2 bands: per head  band0 <- k4 (band 2 of k_sb), band1 <- k3 (band 0)
            for h in range(H):
                for dst_j, src_j in ((0, 2), (1, 0)):
                    dti, dpo = band(h, dst_j)
                    sti, spo = band(h, src_j)
                    nc.vector.tensor_copy(
                        k2_sb[dti][dpo:dpo + G, :], k_sb[sti][spo:spo + G, :])

            # ---- v generation (natural layout + ones columns) ----
            for mt in range(8):
                ps = psA.tile([P, 768], f32, tag="A")
                for half, w in ((0, 512), (512, 256)):
                    for ci in range(CH):
                        nc.tensor.matmul(
                            ps[:, half:half + w],
                            lhsT=xt[ci][:, P * mt:P * (mt + 1)],
                            rhs=wv[ci][:, half:half + w],
                            start=(ci == 0), stop=(ci == CH - 1))
                # scatter 24 x 32 cols -> stride-33 slots
                src = ps[:].rearrange("p (g d) -> p g d", d=32)
                dst = v_sb[mt][:].rearrange("p (g d) -> p g d", d=33)[:, :, 0:32]
                nc.vector.tensor_copy(dst, src)
                ones = v_sb[mt][:].rearrange("p (g d) -> p g d", d=33)[:, :, 32:33]
                nc.vector.memset(ones, 1.0)

            # ---- attention per head ----
            # score blocks as (k tensor, band j, q band j2):
            #   wave1 (k_sb):  j0:(k3,q4)=Sa  j1:(k5,q5)=Sd  j2:(k4,q5)=Se
            #   wave2 (k2_sb): j0:(k4,q4)=Sb  j1:(k3,q5)=Sc
            # AV products accumulate into:
            #   y3 += Sa@[v3|1], Sb@[v4|1];  T = Sd@[v5|1]
            #   y4 += Sc@[v3|1] (+T);        y5 += Se@[v4|1] (+T)
            for h in range(H):
                # --- score matmuls, band-interleaved so 3 (then 2) row
                # tiles of the PE array co-execute; psum tiles pack two
                # consecutive (block, m-tile) results -> one exp each.
                e_map = {}

                def st_wave(blocks):
                    if ST_INTERLEAVE:
                        seq = [(name, ksrc, kj, qj, mt)
                               for mt in range(8)
                               for name, ksrc, kj, qj in blocks]
                    else:
                        seq = [(name, ksrc, kj, qj, mt)
                               for name, ksrc, kj, qj in blocks
                               for mt in range(8)]
                    ps = None
                    for s, (name, ksrc, kj, qj, mt) in enumerate(seq):
                        half = s % 2
                        if half == 0:
                            ps = psA.tile([P, 2 * NQ], f32, tag="A")
                        kti, kpo = band(h, kj)
                        qti, qpo = band(h, qj)
                        nc.tensor.matmul(
                            ps[:, NQ * half:NQ * (half + 1)],
                            lhsT=ksrc[kti][kpo:kpo + G, P * mt:P * (mt + 1)],
                            rhs=q_sb[qti][qpo:qpo + G, :],
                            start=True, stop=True,
                            tile_position=(kpo, 0))
                        e_map[(name, mt)] = (None, half)  # placeholder
                        if half == 1:
                            et = epool.tile([P, 2 * NQ], bf16, tag="e")
                            nc.scalar.activation(
                                et[:], ps[:], mybir.ActivationFunctionType.Exp,
                                scale=SCALE)
                            # backfill the two entries of this tile
                            for nm, mtt in [k for k, v in e_map.items()
                                            if v[0] is None]:
                                e_map[(nm, mtt)] = (et, e_map[(nm, mtt)][1])

                st_wave([("a", k_sb, 0, 0), ("d", k_sb, 1, 1),
                         ("e", k_sb, 2, 2)])
                st_wave([("b", k2_sb, 0, 0), ("c", k2_sb, 1, 1)])

                def e_rhs(name, mt):
                    et, half = e_map[(name, mt)]
                    return et[:, NQ * half:NQ * (half + 1)]

                # --- AV matmuls, column-tiled: stream0 -> PE cols 0-63
                # (psum parts 0-32), stream1 -> cols 64-127 (parts 64-96).
                ps_y3 = psB.tile([P, NQ], f32, tag="av")
                ps_y4 = psB.tile([P, NQ], f32, tag="av")
                ps_t = psB.tile([P, NQ], f32, tag="av")
                ps_y5 = psB.tile([P, NQ], f32, tag="av")
                s0 = ([("a", 3 * h + 0, ps_y3)] * 8 + [("b", 3 * h + 1, ps_y3)] * 8
                      + [("c", 3 * h + 0, ps_y4)] * 8)
                s1 = ([("d", 3 * h + 2, ps_t)] * 8 + [("e", 3 * h + 1, ps_y5)] * 8)
                mt_ctr, started, mm_idx = {}, set(), {}
                counts = {}
                for _, _, ps in s0 + s1:
                    counts[id(ps)] = counts.get(id(ps), 0) + 1
                order = []
                if AV_INTERLEAVE:
                    i0 = i1 = 0
                    while i0 < len(s0) or i1 < len(s1):
                        if i0 < len(s0):
                            order.append((s0[i0], 0)); i0 += 1
                        if i1 < len(s1):
                            order.append((s1[i1], 1)); i1 += 1
                else:
                    order = [(e, 1) for e in s1[:8]] + [(e, 0) for e in s0]                             + [(e, 1) for e in s1[8:]]
                for (name, gg, ps), col in order:
                    mt = mt_ctr.get((id(ps), name), 0)
                    mt_ctr[(id(ps), name)] = mt + 1
                    i = mm_idx.get(id(ps), 0)
                    mm_idx[id(ps)] = i + 1
                    po = 0 if (col == 0 or not AV_COL_TILING) else 64
                    kw = {"tile_position": (0, po)} if AV_COL_TILING else {}
                    nc.tensor.matmul(
                        ps[po:po + 33, :],
                        lhsT=v_sb[mt][:, 33 * gg:33 * gg + 33],
                        rhs=e_rhs(name, mt),
                        start=(i == 0), stop=(i == counts[id(ps)] - 1),
                        **kw)

                # --- normalize: u = y[0:32] * (1/Z), Z = row 32 ---
                t_sb = small.tile([33, NQ], f32, tag="tsb")
                tpo = 64 if AV_COL_TILING else 0
                nc.vector.tensor_copy(t_sb[:], ps_t[tpo:tpo + 33, :])
                ysum4 = small.tile([33, NQ], f32, tag="ysum4")
                nc.vector.tensor_add(ysum4[:], ps_y4[0:33, :], t_sb[:])
                ysum5 = small.tile([33, NQ], f32, tag="ysum5")
                nc.vector.tensor_add(ysum5[:], ps_y5[tpo:tpo + 33, :], t_sb[:])

                zb = small.tile([96, NQ], f32, tag="zb")
                nc.vector.tensor_copy(zb[0:1, :], ps_y3[32:33, :])
                nc.vector.tensor_copy(zb[32:33, :], ysum4[32:33, :])
                nc.vector.tensor_copy(zb[64:65, :], ysum5[32:33, :])
                rz = small.tile([96, NQ], f32, tag="rz")
                nc.vector.reciprocal(rz[:], zb[:])

                for g, ysrc, yslice in ((0, ps_y3, (0, 32)),
                                        (1, ysum4, (0, 32)),
                                        (2, ysum5, (0, 32))):
                    rzb = small.tile([G, NQ], f32, tag="rzb")
                    if g == 0:
                        rzsrc = rz
                    else:
                        rzsrc = small.tile([1, NQ], f32, tag="rzsrc")
                        nc.vector.tensor_copy(rzsrc[:], rz[32 * g:32 * g + 1, :])
                    nc.gpsimd.partition_broadcast(rzb[:], rzsrc[0:1, :])
                    ch = 256 * g + 32 * h
                    nc.vector.tensor_mul(
                        u_sb[ch // P][ch % P:ch % P + G, :],
                        ysrc[yslice[0]:yslice[1], :], rzb[:])

            # ---- projection + bias ----
            for nt in range(4):
                ps = psA.tile([P, C], f32, tag="A")
                for half, w in ((0, 512), (512, 256)):
                    for ci in range(CH):
                        nc.tensor.matmul(
                            ps[:, half:half + w],
                            lhsT=u_sb[ci][:, P * nt:P * (nt + 1)],
                            rhs=wp[ci][:, half:half + w],
                            start=(ci == 0), stop=(ci == CH - 1))
                o_sb = outp.tile([P, C], f32, tag="osb")
                nc.vector.tensor_add(o_sb[:], ps[:], bias[:])
                nc.sync.dma_start(out_d[P * nt:P * (nt + 1), :], o_sb[:])

    nc.finalize()
    return nc


def _prep_inputs(x, W_qkv, W_proj, b_proj):
    bf16 = ml_dtypes.bfloat16
    # wq: per head [q4, q5, q5] (96 cols); wk: per head [k3, k5, k4]
    qcols, kcols = [], []
    for h in range(H):
        qb, kb = HD * h, C + HD * h
        qcols += list(range(qb + 32, qb + 64)) + 2 * list(range(qb + 64, qb + 96))
        kcols += (list(range(kb, kb + 32)) + list(range(kb + 64, kb + 96))
                  + list(range(kb + 32, kb + 64)))
    wq = np.ascontiguousarray(W_qkv[:, qcols]).astype(bf16)
    wk = np.ascontiguousarray(W_qkv[:, kcols]).astype(bf16)
    wv = np.ascontiguousarray(W_qkv[:, 2 * C:3 * C]).astype(bf16)
    wp = np.ascontiguousarray(W_proj).astype(bf16)
    bias = np.broadcast_to(np.asarray(b_proj, np.float32), (P, C)).copy()

    in_maps = []
    for core in range(NCORES):
        b, half = core // 2, core % 2
        xb = np.asarray(x[b], np.float32)
        xp = np.concatenate([xb[NQ * half:NQ * (half + 1)],
                             xb[NQ * (1 - half):NQ * (2 - half)]], axis=0)
        xt = np.ascontiguousarray(xp.T).astype(bf16)
        in_maps.append({"xt": xt, "wq": wq, "wk": wk, "wv": wv, "wp": wp,
                        "bias": bias})
    return in_maps


def kernel(x, W_qkv, W_proj, b_proj, t_h=None, t_w=None, s_h=None, s_w=None,
           **_unused):
    from concourse.bass_utils import run_bass_kernel_spmd

    if "nc" not in _CACHE:
        _CACHE["nc"] = _build_graph()
    nc = _CACHE["nc"]

    in_maps = _prep_inputs(np.asarray(x), np.asarray(W_qkv),
                           np.asarray(W_proj), np.asarray(b_proj))
    res = run_bass_kernel_spmd(nc, in_maps, core_ids=list(range(NCORES)))
    _CACHE["last_results"] = res

    out = np.empty((B, N, C), np.float32)
    for core in range(NCORES):
        b, half = core // 2, core % 2
        out[b, NQ * half:NQ * (half + 1), :] = res.results[core]["out"]
    return out


# revision 18
# speedup vs baseline: 1.0114x; 1.0114x over previous
"""Trainium2 Bass kernel for the 3-group sparse attention module.

Shapes: x [4, 1024, 768], H=8 heads, head_dim 96 split into 3 groups of 32.
  qkv = x @ W_qkv -> q,k,v [B,H,N,96]; groups q3..q5/k3..k5/v3..v5 (32 each)
  x3 = attend(q4, [k3,k4], [v3,v4]); x4 = attend(q5, [k3,k5], [v3,v5])
  x5 = attend(q5, [k4,k5], [v4,v5]);  out = [x3|x4|x5] @ W_proj + b_proj
  scale = 96 ** -0.5

Sharding: 8 cores = 4 batches x 2 query-halves (no collectives).  Each core
computes k/v for the full sequence of its batch (all 8 heads) but queries /
attention / projection only for its 512 rows.  Host passes x transposed
(bf16) with the core's query rows first, so the SPMD graph is identical on
every core; key/value row order is consistently permuted which leaves
attention outputs unchanged.

Everything on-chip runs in "transposed activation space":
  qT/kT[d, n] from matmul(lhsT=W chunk, rhs=xT);  v[m, d] natural.
  S^T[m, n] = matmul(lhsT=kT[32, m-tile], rhs=qT[32, nq]) -- K=32 row-tiled.
  E = exp(scale * S^T) on ScalarE straight out of PSUM (scores are provably
  small: |s*scale| < ~1.2, so no max-subtraction pass is needed).
  y^T[d, n] = matmul(lhsT=[v|1][m-tile, 33], rhs=E) accumulated over m;
  row 32 of the PSUM then holds the softmax denominator Z for free.
  exp(q5 k5^T) @ [v5|1] is shared between x4 and x5 (computed once).
  proj: out[n, :] = matmul(lhsT=yT chunk, rhs=W_proj chunk) + bias.
"""

import numpy as np
import ml_dtypes

B, N, C, H = 4, 1024, 768, 8
HD = 96          # head dim
G = 32           # group dim
NQ = 512         # query rows per core
SCALE = float(HD) ** -0.5
P = 128
NCORES = 8

_CACHE = {}
AV_COL_TILING = False
AV_INTERLEAVE = False
ST_INTERLEAVE = False


def _build_graph():
    import concourse.bass as bass
    import concourse.tile as tile
    from concourse import bacc, mybir

    f32 = mybir.dt.float32
    bf16 = mybir.dt.bfloat16

    nc = bacc.Bacc()

    xt_d = nc.declare_dram_parameter("xt", [C, N], bf16, isOutput=False)
    wq_d = nc.declare_dram_parameter("wq", [C, 768], bf16, isOutput=False)
    wk_d = nc.declare_dram_parameter("wk", [C, 768], bf16, isOutput=False)
    wv_d = nc.declare_dram_parameter("wv", [C, 768], bf16, isOutput=False)
    wp_d = nc.declare_dram_parameter("wp", [C, C], bf16, isOutput=False)
    bias_d = nc.declare_dram_parameter("bias", [P, C], f32, isOutput=False)
    out_d = nc.declare_dram_parameter("out", [NQ, C], f32, isOutput=True)

    CH = C // P  # 6 chunks of 128 along the contraction/channel dims

    with tile.TileContext(nc) as tc:
        with (
            tc.tile_pool(name="wgt", bufs=1) as wgt,
            tc.tile_pool(name="acts", bufs=1) as acts,
            tc.tile_pool(name="epool", bufs=24) as epool,
            tc.tile_pool(name="small", bufs=2) as small,
            tc.tile_pool(name="outp", bufs=2) as outp,
            tc.tile_pool(name="psA", bufs=2, space="PSUM") as psA,
            tc.tile_pool(name="psB", bufs=4, space="PSUM") as psB,
        ):
            # ---- stage inputs in SBUF ----
            xt = [wgt.tile([P, N], bf16, name=f"xt{i}") for i in range(CH)]
            wq = [wgt.tile([P, 768], bf16, name=f"wq{i}") for i in range(CH)]
            wk = [wgt.tile([P, 768], bf16, name=f"wk{i}") for i in range(CH)]
            wv = [wgt.tile([P, 768], bf16, name=f"wv{i}") for i in range(CH)]
            wp = [wgt.tile([P, C], bf16, name=f"wp{i}") for i in range(CH)]
            bias = wgt.tile([P, C], f32, name="bias")
            for i in range(CH):
                nc.sync.dma_start(xt[i][:], xt_d[P * i:P * (i + 1), :])
                nc.sync.dma_start(wq[i][:], wq_d[P * i:P * (i + 1), :])
                nc.sync.dma_start(wk[i][:], wk_d[P * i:P * (i + 1), :])
                nc.sync.dma_start(wv[i][:], wv_d[P * i:P * (i + 1), :])
                nc.sync.dma_start(wp[i][:], wp_d[P * i:P * (i + 1), :])
            nc.sync.dma_start(bias[:], bias_d[:])

            # ---- persistent activation tensors ----
            # qT: [768, 512]  per head h (96 rows at 96h): [q4; q5; q5]
            q_sb = [acts.tile([P, NQ], bf16, name=f"q{i}") for i in range(CH)]
            # kT: [768, 1024] per head: [k3; k5; k4]
            k_sb = [acts.tile([P, N], bf16, name=f"k{i}") for i in range(CH)]
            # kT copy #2 per head: [k4; k3; -] (for the wave-2 score blocks)
            k2_sb = [acts.tile([P, N], bf16, name=f"k2_{i}") for i in range(CH)]
            # v natural per m-tile: 24 groups of [v_g | 1] (33 cols each)
            v_sb = [acts.tile([P, 24 * 33], bf16, name=f"v{i}") for i in range(8)]
            # unnormalized y^T (bf16) channels: 256*g + 32h + d
            u_sb = [acts.tile([P, NQ], bf16, name=f"u{i}") for i in range(CH)]

            def band(h, j):
                """(tensor index, partition offset) of 32-row band j of head h."""
                p = 96 * h + 32 * j
                return p // P, p % P

            # ---- generation helpers (emitted piecemeal, interleaved with
            # attention so ScalarE starts exp-ing as early as possible) ----
            def gen_q(co):
                ps = psA.tile([P, NQ], f32, tag="A")
                for ci in range(CH):
                    nc.tensor.matmul(
                        ps[:], lhsT=wq[ci][:, P * co:P * (co + 1)],
                        rhs=xt[ci][:, 0:NQ],
                        start=(ci == 0), stop=(ci == CH - 1))
                nc.vector.tensor_copy(q_sb[co][:], ps[:])

            def gen_k(co):
                for nh in range(2):
                    ps = psA.tile([P, NQ], f32, tag="A")
                    for ci in range(CH):
                        nc.tensor.matmul(
                            ps[:], lhsT=wk[ci][:, P * co:P * (co + 1)],
                            rhs=xt[ci][:, NQ * nh:NQ * (nh + 1)],
                            start=(ci == 0), stop=(ci == CH - 1))
                    nc.vector.tensor_copy(k_sb[co][:, NQ * nh:NQ * (nh + 1)], ps[:])

            def gen_k2(h):
                # band0 <- k4 (band 2 of k_sb), band1 <- k3 (band 0)
                for dst_j, src_j in ((0, 2), (1, 0)):
                    dti, dpo = band(h, dst_j)
                    sti, spo = band(h, src_j)
                    nc.vector.tensor_copy(
                        k2_sb[dti][dpo:dpo + G, :], k_sb[sti][spo:spo + G, :])

            def gen_v(mt):
                ps = psA.tile([P, 768], f32, tag="A")
                for half, w in ((0, 512), (512, 256)):
                    for ci in range(CH):
                        nc.tensor.matmul(
                            ps[:, half:half + w],
                            lhsT=xt[ci][:, P * mt:P * (mt + 1)],
                            rhs=wv[ci][:, half:half + w],
                            start=(ci == 0), stop=(ci == CH - 1))
                # scatter 24 x 32 cols -> stride-33 slots
                src = ps[:].rearrange("p (g d) -> p g d", d=32)
                dst = v_sb[mt][:].rearrange("p (g d) -> p g d", d=33)[:, :, 0:32]
                nc.vector.tensor_copy(dst, src)
                ones = v_sb[mt][:].rearrange("p (g d) -> p g d", d=33)[:, :, 32:33]
                nc.vector.memset(ones, 1.0)

            # minimal prologue: q/k for head 0, then v while head-0 scores
            # are exp-ing on ScalarE; remaining q/k chunks trickle in
            # between later heads (head h needs q/k tiles <= (96h+95)//128).
            gen_q(0)
            gen_k(0)
            gen_k2(0)
            # gen work scheduled after the ST wave of head h (head h+1's ST
            # only needs q/k tiles <= (96(h+1)+95)//128, all satisfied):
            def post_st0():
                gen_q(1); gen_k(1); gen_k2(1)
                for mt in range(8):
                    gen_v(mt)

            post_st = {0: post_st0,
                       1: lambda: (gen_q(2), gen_k(2), gen_k2(2)),
                       2: lambda: (gen_q(3), gen_k(3), gen_k2(3)),
                       3: lambda: (gen_q(4), gen_k(4), gen_k2(4), gen_k2(5)),
                       4: lambda: (gen_q(5), gen_k(5), gen_k2(6), gen_k2(7))}

            # ---- attention per head ----
            # score blocks as (k tensor, band j, q band j2):
            #   wave1 (k_sb):  j0:(k3,q4)=Sa  j1:(k5,q5)=Sd  j2:(k4,q5)=Se
            #   wave2 (k2_sb): j0:(k4,q4)=Sb  j1:(k3,q5)=Sc
            # AV products accumulate into:
            #   y3 += Sa@[v3|1], Sb@[v4|1];  T = Sd@[v5|1]
            #   y4 += Sc@[v3|1] (+T);        y5 += Se@[v4|1] (+T)
            for h in range(H):
                # --- score matmuls, band-interleaved so 3 (then 2) row
                # tiles of the PE array co-execute; psum tiles pack two
                # consecutive (block, m-tile) results -> one exp each.
                e_map = {}

                def st_wave(blocks):
                    if ST_INTERLEAVE:
                        seq = [(name, ksrc, kj, qj, mt)
                               for mt in range(8)
                               for name, ksrc, kj, qj in blocks]
                    else:
                        seq = [(name, ksrc, kj, qj, mt)
                               for name, ksrc, kj, qj in blocks
                               for mt in range(8)]
                    ps = None
                    for s, (name, ksrc, kj, qj, mt) in enumerate(seq):
                        half = s % 2
                        if half == 0:
                            ps = psA.tile([P, 2 * NQ], f32, tag="A")
                        kti, kpo = band(h, kj)
                        qti, qpo = band(h, qj)
                        nc.tensor.matmul(
                            ps[:, NQ * half:NQ * (half + 1)],
                            lhsT=ksrc[kti][kpo:kpo + G, P * mt:P * (mt + 1)],
                            rhs=q_sb[qti][qpo:qpo + G, :],
                            start=True, stop=True,
                            tile_position=(kpo, 0))
                        e_map[(name, mt)] = (None, half)  # placeholder
                        if half == 1:
                            et = epool.tile([P, 2 * NQ], bf16, tag="e")
                            nc.scalar.activation(
                                et[:], ps[:], mybir.ActivationFunctionType.Exp,
                                scale=SCALE)
                            # backfill the two entries of this tile
                            for nm, mtt in [k for k, v in e_map.items()
                                            if v[0] is None]:
                                e_map[(nm, mtt)] = (et, e_map[(nm, mtt)][1])

                st_wave([("a", k_sb, 0, 0), ("d", k_sb, 1, 1),
                         ("e", k_sb, 2, 2)])
                st_wave([("b", k2_sb, 0, 0), ("c", k2_sb, 1, 1)])

                if h in post_st:
                    post_st[h]()

                def e_rhs(name, mt):
                    et, half = e_map[(name, mt)]
                    return et[:, NQ * half:NQ * (half + 1)]

                # --- AV matmuls, column-tiled: stream0 -> PE cols 0-63
                # (psum parts 0-32), stream1 -> cols 64-127 (parts 64-96).
                ps_y3 = psB.tile([P, NQ], f32, tag="av")
                ps_y4 = psB.tile([P, NQ], f32, tag="av")
                ps_t = psB.tile([P, NQ], f32, tag="av")
                ps_y5 = psB.tile([P, NQ], f32, tag="av")
                s0 = ([("a", 3 * h + 0, ps_y3)] * 8 + [("b", 3 * h + 1, ps_y3)] * 8
                      + [("c", 3 * h + 0, ps_y4)] * 8)
                s1 = ([("d", 3 * h + 2, ps_t)] * 8 + [("e", 3 * h + 1, ps_y5)] * 8)
                mt_ctr, started, mm_idx = {}, set(), {}
                counts = {}
                for _, _, ps in s0 + s1:
                    counts[id(ps)] = counts.get(id(ps), 0) + 1
                order = []
                if AV_INTERLEAVE:
                    i0 = i1 = 0
                    while i0 < len(s0) or i1 < len(s1):
                        if i0 < len(s0):
                            order.append((s0[i0], 0)); i0 += 1
                        if i1 < len(s1):
                            order.append((s1[i1], 1)); i1 += 1
                else:
                    order = [(e, 1) for e in s1[:8]] + [(e, 0) for e in s0]                             + [(e, 1) for e in s1[8:]]
                for (name, gg, ps), col in order:
                    mt = mt_ctr.get((id(ps), name), 0)
                    mt_ctr[(id(ps), name)] = mt + 1
                    i = mm_idx.get(id(ps), 0)
                    mm_idx[id(ps)] = i + 1
                    po = 0 if (col == 0 or not AV_COL_TILING) else 64
                    kw = {"tile_position": (0, po)} if AV_COL_TILING else {}
                    nc.tensor.matmul(
                        ps[po:po + 33, :],
                        lhsT=v_sb[mt][:, 33 * gg:33 * gg + 33],
                        rhs=e_rhs(name, mt),
                        start=(i == 0), stop=(i == counts[id(ps)] - 1),
                        **kw)

                # --- normalize: u = y[0:32] * (1/Z), Z = row 32 ---
                t_sb = small.tile([33, NQ], f32, tag="tsb")
                tpo = 64 if AV_COL_TILING else 0
                nc.vector.tensor_copy(t_sb[:], ps_t[tpo:tpo + 33, :])
                ysum4 = small.tile([33, NQ], f32, tag="ysum4")
                nc.vector.tensor_add(ysum4[:], ps_y4[0:33, :], t_sb[:])
                ysum5 = small.tile([33, NQ], f32, tag="ysum5")
                nc.vector.tensor_add(ysum5[:], ps_y5[tpo:tpo + 33, :], t_sb[:])

                zb = small.tile([96, NQ], f32, tag="zb")
                nc.vector.tensor_copy(zb[0:1, :], ps_y3[32:33, :])
                nc.vector.tensor_copy(zb[32:33, :], ysum4[32:33, :])
                nc.vector.tensor_copy(zb[64:65, :], ysum5[32:33, :])
                rz = small.tile([96, NQ], f32, tag="rz")
                nc.vector.reciprocal(rz[:], zb[:])

                for g, ysrc, yslice in ((0, ps_y3, (0, 32)),
                                        (1, ysum4, (0, 32)),
                                        (2, ysum5, (0, 32))):
                    rzb = small.tile([G, NQ], f32, tag="rzb")
                    if g == 0:
                        rzsrc = rz
                    else:
                        rzsrc = small.tile([1, NQ], f32, tag="rzsrc")
                        nc.vector.tensor_copy(rzsrc[:], rz[32 * g:32 * g + 1, :])
                    nc.gpsimd.partition_broadcast(rzb[:], rzsrc[0:1, :])
                    ch = 256 * g + 32 * h
                    nc.vector.tensor_mul(
                        u_sb[ch // P][ch % P:ch % P + G, :],
                        ysrc[yslice[0]:yslice[1], :], rzb[:])

            # ---- projection + bias ----
            for nt in range(4):
                ps = psA.tile([P, C], f32, tag="A")
                for half, w in ((0, 512), (512, 256)):
                    for ci in range(CH):
                        nc.tensor.matmul(
                            ps[:, half:half + w],
                            lhsT=u_sb[ci][:, P * nt:P * (nt + 1)],
                            rhs=wp[ci][:, half:half + w],
                            start=(ci == 0), stop=(ci == CH - 1))
                o_sb = outp.tile([P, C], f32, tag="osb")
                nc.vector.tensor_add(o_sb[:], ps[:], bias[:])
                nc.sync.dma_start(out_d[P * nt:P * (nt + 1), :], o_sb[:])

    nc.finalize()
    return nc


def _prep_inputs(x, W_qkv, W_proj, b_proj):
    bf16 = ml_dtypes.bfloat16
    # wq: per head [q4, q5, q5] (96 cols); wk: per head [k3, k5, k4]
    qcols, kcols = [], []
    for h in range(H):
        qb, kb = HD * h, C + HD * h
        qcols += list(range(qb + 32, qb + 64)) + 2 * list(range(qb + 64, qb + 96))
        kcols += (list(range(kb, kb + 32)) + list(range(kb + 64, kb + 96))
                  + list(range(kb + 32, kb + 64)))
    wq = np.ascontiguousarray(W_qkv[:, qcols]).astype(bf16)
    wk = np.ascontiguousarray(W_qkv[:, kcols]).astype(bf16)
    wv = np.ascontiguousarray(W_qkv[:, 2 * C:3 * C]).astype(bf16)
    wp = np.ascontiguousarray(W_proj).astype(bf16)
    bias = np.broadcast_to(np.asarray(b_proj, np.float32), (P, C)).copy()

    in_maps = []
    for core in range(NCORES):
        b, half = core // 2, core % 2
        xb = np.asarray(x[b], np.float32)
        xp = np.concatenate([xb[NQ * half:NQ * (half + 1)],
                             xb[NQ * (1 - half):NQ * (2 - half)]], axis=0)
        xt = np.ascontiguousarray(xp.T).astype(bf16)
        in_maps.append({"xt": xt, "wq": wq, "wk": wk, "wv": wv, "wp": wp,
                        "bias": bias})
    return in_maps


def kernel(x, W_qkv, W_proj, b_proj, t_h=None, t_w=None, s_h=None, s_w=None,
           **_unused):
    from concourse.bass_utils import run_bass_kernel_spmd

    if "nc" not in _CACHE:
        _CACHE["nc"] = _build_graph()
    nc = _CACHE["nc"]

    in_maps = _prep_inputs(np.asarray(x), np.asarray(W_qkv),
                           np.asarray(W_proj), np.asarray(b_proj))
    res = run_bass_kernel_spmd(nc, in_maps, core_ids=list(range(NCORES)))
    _CACHE["last_results"] = res

    out = np.empty((B, N, C), np.float32)
    for core in range(NCORES):
        b, half = core // 2, core % 2
        out[b, NQ * half:NQ * (half + 1), :] = res.results[core]["out"]
    return out


# revision 30
# speedup vs baseline: 1.2155x; 1.2018x over previous
"""Trainium2 Bass kernel for the 3-group sparse attention module.

Shapes: x [4, 1024, 768], H=8 heads, head_dim 96 split into 3 groups of 32.
  qkv = x @ W_qkv -> q,k,v [B,H,N,96]; groups q3..q5/k3..k5/v3..v5 (32 each)
  x3 = attend(q4, [k3,k4], [v3,v4]); x4 = attend(q5, [k3,k5], [v3,v5])
  x5 = attend(q5, [k4,k5], [v4,v5]);  out = [x3|x4|x5] @ W_proj + b_proj
  scale = 96 ** -0.5

Sharding: 8 cores = 4 batches x 2 query-halves (no collectives).  Each core
computes k/v for the full sequence of its batch (all 8 heads) but queries /
attention / projection only for its 512 rows.  Host passes x transposed
(bf16) with the core's query rows first, so the SPMD graph is identical on
every core; key/value row order is consistently permuted which leaves
attention outputs unchanged.

Everything on-chip runs in "transposed activation space":
  qT/kT[d, n] from matmul(lhsT=W chunk, rhs=xT);  v[m, d] natural.
  S^T[m, n] = matmul(lhsT=kT[32, m-tile], rhs=qT[32, nq]) -- K=32 row-tiled.
  E = exp(scale * S^T) on ScalarE straight out of PSUM (scores are provably
  small: |s*scale| < ~1.2, so no max-subtraction pass is needed).
  y^T[d, n] = matmul(lhsT=[v|1][m-tile, 33], rhs=E) accumulated over m;
  row 32 of the PSUM then holds the softmax denominator Z for free.
  exp(q5 k5^T) @ [v5|1] is shared between x4 and x5 (computed once).
  proj: out[n, :] = matmul(lhsT=yT chunk, rhs=W_proj chunk) + bias.
"""

import numpy as np
import ml_dtypes

B, N, C, H = 4, 1024, 768, 8
HD = 96          # head dim
G = 32           # group dim
NQ = 512         # query rows per core
SCALE = float(HD) ** -0.5
P = 128
NCORES = 8

_CACHE = {}
# tuned configuration (measured best on TRN2):
AV_COL_TILING = False   # PE col-tiling for AV: slower (mode-switch drains)
E_FP8 = False           # fp8 probabilities fail the accuracy budget
AV_INTERLEAVE = True    # interleave the two AV accumulation streams
ST_INTERLEAVE = True    # round-robin score matmuls across PE row-bands


def _build_graph():
    import concourse.bass as bass
    import concourse.tile as tile
    from concourse import bacc, mybir

    f32 = mybir.dt.float32
    bf16 = mybir.dt.bfloat16
    edt = mybir.dt.float8e4 if E_FP8 else mybir.dt.bfloat16

    nc = bacc.Bacc()

    xt_d = nc.declare_dram_parameter("xt", [C, N], bf16, isOutput=False)
    wq_d = nc.declare_dram_parameter("wq", [C, 768], bf16, isOutput=False)
    wk_d = nc.declare_dram_parameter("wk", [C, 768], bf16, isOutput=False)
    wv_d = nc.declare_dram_parameter("wv", [C, 768], bf16, isOutput=False)
    wp_d = nc.declare_dram_parameter("wp", [C, C], bf16, isOutput=False)
    bias_d = nc.declare_dram_parameter("bias", [P, C], f32, isOutput=False)
    out_d = nc.declare_dram_parameter("out", [NQ, C], f32, isOutput=True)

    CH = C // P  # 6 chunks of 128 along the contraction/channel dims

    with tile.TileContext(nc) as tc:
        with (
            tc.tile_pool(name="wgt", bufs=1) as wgt,
            tc.tile_pool(name="acts", bufs=1) as acts,
            tc.tile_pool(name="epool", bufs=30) as epool,
            tc.tile_pool(name="small", bufs=2) as small,
            tc.tile_pool(name="outp", bufs=2) as outp,
            tc.tile_pool(name="psA", bufs=2, space="PSUM") as psA,
            tc.tile_pool(name="psB", bufs=4, space="PSUM") as psB,
        ):
            # ---- stage inputs in SBUF ----
            xt = [wgt.tile([P, N], bf16, name=f"xt{i}") for i in range(CH)]
            wq = [wgt.tile([P, 768], bf16, name=f"wq{i}") for i in range(CH)]
            wk = [wgt.tile([P, 768], bf16, name=f"wk{i}") for i in range(CH)]
            wv = [wgt.tile([P, 768], bf16, name=f"wv{i}") for i in range(CH)]
            wp = [wgt.tile([P, C], bf16, name=f"wp{i}") for i in range(CH)]
            bias = wgt.tile([P, C], f32, name="bias")
            for i in range(CH):
                nc.sync.dma_start(xt[i][:], xt_d[P * i:P * (i + 1), :])
                nc.sync.dma_start(wq[i][:], wq_d[P * i:P * (i + 1), :])
                nc.sync.dma_start(wk[i][:], wk_d[P * i:P * (i + 1), :])
            for i in range(CH):
                nc.sync.dma_start(wv[i][:], wv_d[P * i:P * (i + 1), :])
                nc.sync.dma_start(wp[i][:], wp_d[P * i:P * (i + 1), :])
            nc.sync.dma_start(bias[:], bias_d[:])

            # ---- persistent activation tensors ----
            # qT: [768, 512]  per head h (96 rows at 96h): [q4; q5; q5]
            q_sb = [acts.tile([P, NQ], bf16, name=f"q{i}") for i in range(CH)]
            # kT: [768, 1024] per head: [k3; k5; k4]
            k_sb = [acts.tile([P, N], bf16, name=f"k{i}") for i in range(CH)]
            # qT copy #2 per head: band0 <- q5, band2 <- q4 (wave-2 blocks)
            q2_sb = [acts.tile([P, NQ], bf16, name=f"q2_{i}") for i in range(CH)]
            # v natural per m-tile: 24 groups of [v_g | 1] (33 cols each)
            v_sb = [acts.tile([P, 24 * 33], bf16, name=f"v{i}") for i in range(8)]
            # unnormalized y^T (bf16) channels: 256*g + 32h + d
            u_sb = [acts.tile([P, NQ], bf16, name=f"u{i}") for i in range(CH)]

            def band(h, j):
                """(tensor index, partition offset) of 32-row band j of head h."""
                p = 96 * h + 32 * j
                return p // P, p % P

            # ---- generation helpers (emitted piecemeal, interleaved with
            # attention so ScalarE starts exp-ing as early as possible) ----
            def gen_q(co):
                ps = psA.tile([P, NQ], f32, tag="A")
                for ci in range(CH):
                    nc.tensor.matmul(
                        ps[:], lhsT=wq[ci][:, P * co:P * (co + 1)],
                        rhs=xt[ci][:, 0:NQ],
                        start=(ci == 0), stop=(ci == CH - 1))
                nc.vector.tensor_copy(q_sb[co][:], ps[:])

            def gen_k(co):
                for nh in range(2):
                    ps = psA.tile([P, NQ], f32, tag="A")
                    for ci in range(CH):
                        nc.tensor.matmul(
                            ps[:], lhsT=wk[ci][:, P * co:P * (co + 1)],
                            rhs=xt[ci][:, NQ * nh:NQ * (nh + 1)],
                            start=(ci == 0), stop=(ci == CH - 1))
                    nc.vector.tensor_copy(k_sb[co][:, NQ * nh:NQ * (nh + 1)], ps[:])

            def gen_q2(h):
                # band2 <- q4 (q_sb band 0), band0 <- q5 (q_sb band 1)
                for dst_j, src_j in ((2, 0), (0, 1)):
                    dti, dpo = band(h, dst_j)
                    sti, spo = band(h, src_j)
                    nc.vector.tensor_copy(
                        q2_sb[dti][dpo:dpo + G, :], q_sb[sti][spo:spo + G, :])

            def gen_v(mt):
                ps = psA.tile([P, 768], f32, tag="A")
                for half, w in ((0, 512), (512, 256)):
                    for ci in range(CH):
                        nc.tensor.matmul(
                            ps[:, half:half + w],
                            lhsT=xt[ci][:, P * mt:P * (mt + 1)],
                            rhs=wv[ci][:, half:half + w],
                            start=(ci == 0), stop=(ci == CH - 1))
                vdst = v_sb[mt][:].rearrange("p (g d) -> p g d", d=33)
                nc.vector.tensor_copy(
                    vdst[:, :, 0:32], ps[:].rearrange("p (g d) -> p g d", d=32))
                nc.vector.memset(vdst[:, :, 32:33], 1.0)

            # prologue: just enough for head 0; everything else is emitted
            # AFTER the attention chain (= lower scheduler priority) so the
            # list scheduler uses it as PE filler whenever attention stalls.
            gen_q(0)
            gen_k(0)
            gen_q2(0)

            with tc.high_priority(offset=-1000000):
                gen_q(1); gen_k(1); gen_q2(1)
                for mt in range(8):
                    gen_v(mt)
                for co in range(2, CH):
                    gen_q(co); gen_k(co)
                for h2 in range(2, H):
                    gen_q2(h2)

            # ---- attention per head ----
            # score blocks as (k tensor, band j, q band j2):
            #   wave1 (k_sb):  j0:(k3,q4)=Sa  j1:(k5,q5)=Sd  j2:(k4,q5)=Se
            #   wave2 (k2_sb): j0:(k4,q4)=Sb  j1:(k3,q5)=Sc
            # AV products accumulate into:
            #   y3 += Sa@[v3|1], Sb@[v4|1];  T = Sd@[v5|1]
            #   y4 += Sc@[v3|1] (+T);        y5 += Se@[v4|1] (+T)
            for h in range(H):
                # --- score matmuls, band-interleaved so 3 (then 2) row
                # tiles of the PE array co-execute; psum tiles pack two
                # consecutive (block, m-tile) results -> one exp each.
                e_map = {}

                def st_wave(blocks):
                    if ST_INTERLEAVE:
                        seq = [(name, ksrc, kj, qj, qsrc, mt)
                               for mt in range(8)
                               for name, ksrc, kj, qj, qsrc in blocks]
                    else:
                        seq = [(name, ksrc, kj, qj, qsrc, mt)
                               for name, ksrc, kj, qj, qsrc in blocks
                               for mt in range(8)]
                    ps = None
                    for s, (name, ksrc, kj, qj, qsrc, mt) in enumerate(seq):
                        half = s % 2
                        if half == 0:
                            ps = psA.tile([P, 2 * NQ], f32, tag="A")
                        kti, kpo = band(h, kj)
                        qti, qpo = band(h, qj)
                        nc.tensor.matmul(
                            ps[:, NQ * half:NQ * (half + 1)],
                            lhsT=ksrc[kti][kpo:kpo + G, P * mt:P * (mt + 1)],
                            rhs=qsrc[qti][qpo:qpo + G, :],
                            start=True, stop=True,
                            tile_position=(kpo, 0))
                        e_map[(name, mt)] = (None, half)  # placeholder
                        if half == 1:
                            et = epool.tile([P, 2 * NQ], edt, tag="e")
                            nc.scalar.activation(
                                et[:], ps[:], mybir.ActivationFunctionType.Exp,
                                scale=SCALE)
                            # backfill the two entries of this tile
                            for nm, mtt in [k for k, v in e_map.items()
                                            if v[0] is None]:
                                e_map[(nm, mtt)] = (et, e_map[(nm, mtt)][1])

                st_wave([("a", k_sb, 0, 0, q_sb), ("d", k_sb, 1, 1, q_sb),
                         ("e", k_sb, 2, 2, q_sb)])
                st_wave([("b", k_sb, 2, 2, q2_sb), ("c", k_sb, 0, 0, q2_sb)])

                def e_rhs(name, mt):
                    et, half = e_map[(name, mt)]
                    return et[:, NQ * half:NQ * (half + 1)]

                # --- AV matmuls, column-tiled: stream0 -> PE cols 0-63
                # (psum parts 0-32), stream1 -> cols 64-127 (parts 64-96).
                ps_y3 = psB.tile([P, NQ], f32, tag="av")
                ps_y4 = psB.tile([P, NQ], f32, tag="av")
                ps_t = psB.tile([P, NQ], f32, tag="av")
                ps_y5 = psB.tile([P, NQ], f32, tag="av")
                s0 = ([("a", 3 * h + 0, ps_y3)] * 8 + [("b", 3 * h + 1, ps_y3)] * 8
                      + [("c", 3 * h + 0, ps_y4)] * 8)
                s1 = ([("d", 3 * h + 2, ps_t)] * 8 + [("e", 3 * h + 1, ps_y5)] * 8)
                mt_ctr, started, mm_idx = {}, set(), {}
                counts = {}
                for _, _, ps in s0 + s1:
                    counts[id(ps)] = counts.get(id(ps), 0) + 1
                order = []
                if AV_INTERLEAVE:
                    i0 = i1 = 0
                    while i0 < len(s0) or i1 < len(s1):
                        if i0 < len(s0):
                            order.append((s0[i0], 0)); i0 += 1
                        if i1 < len(s1):
                            order.append((s1[i1], 1)); i1 += 1
                else:
                    order = [(e, 1) for e in s1[:8]] + [(e, 0) for e in s0]                             + [(e, 1) for e in s1[8:]]
                for (name, gg, ps), col in order:
                    mt = mt_ctr.get((id(ps), name), 0)
                    mt_ctr[(id(ps), name)] = mt + 1
                    i = mm_idx.get(id(ps), 0)
                    mm_idx[id(ps)] = i + 1
                    po = 0 if (col == 0 or not AV_COL_TILING) else 64
                    kw = {"tile_position": (0, po)} if AV_COL_TILING else {}
                    nc.tensor.matmul(
                        ps[po:po + 33, :],
                        lhsT=v_sb[mt][:, 33 * gg:33 * gg + 33],
                        rhs=e_rhs(name, mt),
                        start=(i == 0), stop=(i == counts[id(ps)] - 1),
                        **kw)

                # --- normalize: u = y[0:32] * (1/Z), Z = row 32 ---
                t_sb = small.tile([33, NQ], f32, tag="tsb")
                tpo = 64 if AV_COL_TILING else 0
                nc.vector.tensor_copy(t_sb[:], ps_t[tpo:tpo + 33, :])
                ysum4 = small.tile([33, NQ], f32, tag="ysum4")
                nc.vector.tensor_add(ysum4[:], ps_y4[0:33, :], t_sb[:])
                ysum5 = small.tile([33, NQ], f32, tag="ysum5")
                nc.vector.tensor_add(ysum5[:], ps_y5[tpo:tpo + 33, :], t_sb[:])

                zb = small.tile([96, NQ], f32, tag="zb")
                nc.vector.tensor_copy(zb[0:1, :], ps_y3[32:33, :])
                nc.vector.tensor_copy(zb[32:33, :], ysum4[32:33, :])
                nc.vector.tensor_copy(zb[64:65, :], ysum5[32:33, :])
                rz = small.tile([96, NQ], f32, tag="rz")
                nc.vector.reciprocal(rz[:], zb[:])

                for g, ysrc, yslice in ((0, ps_y3, (0, 32)),
                                        (1, ysum4, (0, 32)),
                                        (2, ysum5, (0, 32))):
                    rzb = small.tile([G, NQ], f32, tag="rzb")
                    if g == 0:
                        rzsrc = rz
                    else:
                        rzsrc = small.tile([1, NQ], f32, tag="rzsrc")
                        nc.vector.tensor_copy(rzsrc[:], rz[32 * g:32 * g + 1, :])
                    nc.gpsimd.partition_broadcast(rzb[:], rzsrc[0:1, :])
                    ch = 256 * g + 32 * h
                    nc.vector.tensor_mul(
                        u_sb[ch // P][ch % P:ch % P + G, :],
                        ysrc[yslice[0]:yslice[1], :], rzb[:])

            # ---- projection + bias ----
            for nt in range(4):
                ps = psA.tile([P, C], f32, tag="A")
                for half, w in ((0, 512), (512, 256)):
                    for ci in range(CH):
                        nc.tensor.matmul(
                            ps[:, half:half + w],
                            lhsT=u_sb[ci][:, P * nt:P * (nt + 1)],
                            rhs=wp[ci][:, half:half + w],
                            start=(ci == 0), stop=(ci == CH - 1))
                o_sb = outp.tile([P, C], f32, tag="osb")
                nc.vector.tensor_add(o_sb[:], ps[:], bias[:])
                nc.sync.dma_start(out_d[P * nt:P * (nt + 1), :], o_sb[:])

    nc.finalize()
    return nc


def _prep_inputs(x, W_qkv, W_proj, b_proj):
    bf16 = ml_dtypes.bfloat16
    # wq: per head [q4, q5, q5] (96 cols); wk: per head [k3, k5, k4]
    qcols, kcols = [], []
    for h in range(H):
        qb, kb = HD * h, C + HD * h
        qcols += list(range(qb + 32, qb + 64)) + 2 * list(range(qb + 64, qb + 96))
        kcols += (list(range(kb, kb + 32)) + list(range(kb + 64, kb + 96))
                  + list(range(kb + 32, kb + 64)))
    wq = np.ascontiguousarray(W_qkv[:, qcols]).astype(bf16)
    wk = np.ascontiguousarray(W_qkv[:, kcols]).astype(bf16)
    wv = np.ascontiguousarray(W_qkv[:, 2 * C:3 * C]).astype(bf16)
    wp = np.ascontiguousarray(W_proj).astype(bf16)
    bias = np.broadcast_to(np.asarray(b_proj, np.float32), (P, C)).copy()

    in_maps = []
    for core in range(NCORES):
        b, half = core // 2, core % 2
        xb = np.asarray(x[b], np.float32)
        xp = np.concatenate([xb[NQ * half:NQ * (half + 1)],
                             xb[NQ * (1 - half):NQ * (2 - half)]], axis=0)
        xt = np.ascontiguousarray(xp.T).astype(bf16)
        in_maps.append({"xt": xt, "wq": wq, "wk": wk, "wv": wv, "wp": wp,
                        "bias": bias})
    return in_maps


def kernel(x, W_qkv, W_proj, b_proj, t_h=None, t_w=None, s_h=None, s_w=None,
           **_unused):
    from concourse.bass_utils import run_bass_kernel_spmd

    if "nc" not in _CACHE:
        _CACHE["nc"] = _build_graph()
    nc = _CACHE["nc"]

    in_maps = _prep_inputs(np.asarray(x), np.asarray(W_qkv),
                           np.asarray(W_proj), np.asarray(b_proj))
    res = run_bass_kernel_spmd(nc, in_maps, core_ids=list(range(NCORES)))
    _CACHE["last_results"] = res

    out = np.empty((B, N, C), np.float32)
    for core in range(NCORES):
        b, half = core // 2, core % 2
        out[b, NQ * half:NQ * (half + 1), :] = res.results[core]["out"]
    return out
